# revision 11
# baseline (speedup 1.0000x reference)
"""PointPillarScatter Trainium2 kernel (v2: fp16 + tile pairing).

Strategy: shard by (batch, y-half) -> 8 cores, each producing a
[64, 107136] channel-major slab of the BEV grid, written as fp16 and
upcast to f32 on the host (quantization error ~2.4e-4 L2, far below the
2e-2 gate).

The scatter+transpose+zero-fill is fused into per-pair PE matmuls. Each
core's slab is split into 420 tiles of 256 cells (tile 419 is padding);
tiles t and t+210 are PAIRED so one matmul produces a full 128-partition
output [2 tiles x 64 channels, 256 cells]:

  psum[128, 256] = lhsT[128, 128]^T @ onehot[128, 256]

lhsT is block-diagonal per pair: partitions 0..63 hold tile-A pillar
features in columns 0..63, partitions 64..127 hold tile-B features in
columns 64..127. onehot[k, j] = (cell_offset_k == j) is built by
is_equal against an iota row. PSUM start=True zeros empty cells.

fp16 keeps everything exact: the one-hot is 0/1, each output cell has at
most one contributing pillar (host dedups last-write-wins), offsets
0..255 are exactly representable, so the only error is the single f32 ->
fp16 feature quantization.

Work is spread across engines: one-hot on DVE (stored offset-major /
pair-minor so every is_equal operand has a packed fp16 innermost dim,
the layout DVE's 2x/4x fast modes need; the matmul reads its one-hot
columns with a small stride instead), PSUM->SBUF fp16-converting copies
split ACT/DVE, feature loads on the ACT HWDGE ring (two strided
64-partition DMAs write the diagonal blocks of pre-zeroed chunk
buffers), output stores batched 10 pairs per DMA on the SP ring (5KB
contiguous per partition).
"""

import numpy as np

B, C, NY, NX = 4, 64, 496, 432
CELLS_B = NY * NX          # 214272 cells per batch
HALF = CELLS_B // 2        # 107136 cells per core slab
N_CORES = 8
TILE = 256                 # cells per tile
P = 210                    # pairs per core; pair q = tiles (q, q+210)
N_T = 2 * P                # 420 tiles (tile 419 is padding)
HALF_PAD = N_T * TILE      # 107520
SLOT = 64                  # pillar slots per tile (max K per 256-cell tile)
GROUP = 10                 # pairs per onehot/stage/output-DMA group (210 = 21*10)
N_GROUPS = P // GROUP      # 21
CHUNK = 30                 # pairs per feature-DMA chunk (210 = 7*30)
N_CHUNKS = P // CHUNK      # 7
PSUB = 4                   # pairs per PSUM tile ([128, 1024] f32 = 2 banks)


def make_iota():
    """[128, TILE*GROUP] f16, interleaved: column j*GROUP + i holds j.

    The one-hot is stored offset-major / pair-minor so that the is_equal
    broadcast lands on a middle AP dim and every operand's innermost dim
    is packed stride-1 fp16 — the layout DVE's 2x/4x fast modes need.
    """
    row = np.repeat(np.arange(TILE, dtype=np.float16), GROUP)
    return np.broadcast_to(row[None, :], (128, TILE * GROUP)).copy()


def _host_prep(pf, vc):
    """Dedup (last-wins), shard, bucket pillars into (core, tile, slot).

    Returns featAB [N_CORES, 128, P*64] f16 (A slots in partitions 0..63
    holding channel columns 0..63 of each pair block... stored compact:
    partition p, pair q, 64 channels), offs [N_CORES, 128, P] f16
    (cell offset within tile, -1 for empty slots), or None if any tile
    has more than SLOT pillars (fallback path).
    """
    pf = np.asarray(pf, dtype=np.float32)
    vc = np.asarray(vc)
    b = vc[:, 0].astype(np.int64)
    y = vc[:, 2].astype(np.int64)
    x = vc[:, 3].astype(np.int64)
    cell = y * NX + x
    key = b * CELLS_B + cell

    # last occurrence of each key wins (matches reference scatter)
    u, idx_rev = np.unique(key[::-1], return_index=True)
    winners = (len(key) - 1) - idx_rev

    wb = u // CELLS_B
    wc = u % CELLS_B
    h = (wc >= HALF).astype(np.int64)
    core = wb * 2 + h
    cl = wc - h * HALF
    t = cl // TILE                     # 0..418
    off = cl % TILE
    half = (t >= P).astype(np.int64)
    q = t - half * P

    # rank within each (core, tile) bucket
    gkey = core * N_T + t
    order = np.argsort(gkey, kind="stable")
    gk_s = gkey[order]
    starts = np.r_[0, np.flatnonzero(np.diff(gk_s)) + 1]
    counts = np.diff(np.r_[starts, len(gk_s)])
    if counts.max() > SLOT:
        return None
    rank = np.arange(len(gk_s)) - np.repeat(starts, counts)

    w_s = winners[order]
    core_s = core[order]
    part_s = half[order] * 64 + rank
    q_s = q[order]
    off_s = off[order]

    feat4 = np.zeros((N_CORES, 128, P, 64), np.float16)
    offs = np.full((N_CORES, 128, P), -1.0, np.float16)
    feat4[core_s, part_s, q_s, :] = pf[w_s].astype(np.float16)
    offs[core_s, part_s, q_s] = off_s.astype(np.float16)
    return feat4.reshape(N_CORES, 128, P * 64), offs


def _build_bass(repeat=1):
    import concourse.bacc as bacc
    import concourse.tile as tile
    from concourse import mybir
    from contextlib import ExitStack

    f16 = mybir.dt.float16
    f32 = mybir.dt.float32
    nc = bacc.Bacc("TRN2", target_bir_lowering=False, debug=False)

    featAB = nc.dram_tensor("featAB", [128, P * 64], f16, kind="ExternalInput")
    offs = nc.dram_tensor("offs", [128, P], f16, kind="ExternalInput")
    iota = nc.dram_tensor("iota", [128, TILE * GROUP], f16, kind="ExternalInput")
    out = nc.dram_tensor("out", [128, P * TILE], f16, kind="ExternalOutput")

    with tile.TileContext(nc) as tc, ExitStack() as ctx:
        const_p = ctx.enter_context(tc.tile_pool(name="const", bufs=1))
        lhsT_p = ctx.enter_context(tc.tile_pool(name="lhsT", bufs=3))
        oh_p = ctx.enter_context(tc.tile_pool(name="oh", bufs=4))
        ps_p = ctx.enter_context(tc.tile_pool(name="ps", bufs=4, space="PSUM"))
        st_p = ctx.enter_context(tc.tile_pool(name="st", bufs=4))

        iota_t = const_p.tile([128, TILE * GROUP], f16)
        nc.sync.dma_start(out=iota_t[:], in_=iota[:, :])
        off_t = const_p.tile([128, P], f16)
        nc.sync.dma_start(out=off_t[:], in_=offs[:, :])

        # Pre-zero all lhsT chunk buffers once: in-loop DMAs only rewrite
        # the diagonal blocks, so the off-diagonal zeros persist across
        # buffer rotations.
        for _ in range(3):
            z = lhsT_p.tile([128, CHUNK, 128], f16, tag="lhsT")
            nc.vector.memset(z[:], 0.0)

        def body():
            chunk_tiles = {}
            state = {"next": 0}
            # greedy engine load balancing (estimated ns per free-element):
            # one-hot runs on DVE (fast-mode eligible); the PSUM->SBUF
            # fp16-converting copies are split between ACT and DVE.
            load = {"dve": 0.0, "act": 0.0}

            def pick(a, b):
                return a if load[a] <= load[b] else b

            def issue_chunks(upto_pair):
                while (
                    state["next"] < N_CHUNKS
                    and state["next"] * CHUNK < upto_pair
                ):
                    c = state["next"]
                    lt = lhsT_p.tile([128, CHUNK, 128], f16, tag="lhsT")
                    lo, hi = c * CHUNK * 64, (c + 1) * CHUNK * 64
                    nc.scalar.dma_start(
                        out=lt[0:64, :, 0:64], in_=featAB[0:64, lo:hi]
                    )
                    nc.scalar.dma_start(
                        out=lt[64:128, :, 64:128], in_=featAB[64:128, lo:hi]
                    )
                    chunk_tiles[c] = lt
                    state["next"] += 1

            for g in range(N_GROUPS):
                q0 = g * GROUP
                n = min(GROUP, P - q0)
                issue_chunks(q0 + n + CHUNK)  # one-chunk lookahead

                # one-hot, offset-major/pair-minor: oh[p, j, i] is 1 iff
                # pillar p of pair q0+i sits at cell offset j. All operands
                # have packed fp16 innermost dims (DVE fast-mode layout).
                oh = oh_p.tile([128, TILE, n], f16, tag="oh")
                load["dve"] += n * TILE * 0.27 + 150
                nc.vector.tensor_tensor(
                    out=oh[:],
                    in0=off_t[:, q0 : q0 + n]
                    .to_broadcast([128, n, TILE])
                    .transpose([0, 2, 1]),
                    in1=iota_t[:, : TILE * n],
                    op=mybir.AluOpType.is_equal,
                )

                stage = st_p.tile([128, n * TILE], f16, tag="st")
                for s0 in range(0, n, PSUB):
                    m = min(PSUB, n - s0)
                    psum = ps_p.tile([128, m * TILE], f32, tag="ps")
                    for j in range(m):
                        qq = q0 + s0 + j
                        c, qc = divmod(qq, CHUNK)
                        nc.tensor.matmul(
                            out=psum[:, j * TILE : (j + 1) * TILE],
                            lhsT=chunk_tiles[c][:, qc, :],
                            rhs=oh[:, :, s0 + j],
                            start=True,
                            stop=True,
                        )
                    w = pick("act", "dve")
                    load[w] += m * TILE * (0.83 if w == "act" else 1.04) + 300
                    if w == "act":
                        nc.scalar.copy(
                            out=stage[:, s0 * TILE : (s0 + m) * TILE],
                            in_=psum[:],
                        )
                    else:
                        nc.vector.tensor_copy(
                            out=stage[:, s0 * TILE : (s0 + m) * TILE],
                            in_=psum[:],
                        )
                nc.sync.dma_start(
                    out=out[:, q0 * TILE : (q0 + n) * TILE], in_=stage[:]
                )

        if repeat == 1:
            body()
        else:
            with tc.For_i(0, repeat, 1):
                body()

    nc.compile()
    return nc


def _sim_core(featAB_c, offs_c):
    """Numpy simulation of one core's device program (for validation)."""
    fv = featAB_c.reshape(128, P, 64).astype(np.float32)
    out = np.zeros((128, P * TILE), np.float32)
    for q in range(P):
        oh = (
            offs_c[:, q : q + 1].astype(np.float32)
            == np.arange(TILE, dtype=np.float32)[None, :]
        ).astype(np.float32)
        lhsT = np.zeros((128, 128), np.float32)
        lhsT[0:64, 0:64] = fv[0:64, q, :]
        lhsT[64:128, 64:128] = fv[64:128, q, :]
        out[:, q * TILE : (q + 1) * TILE] = lhsT.T @ oh
    return out


def _assemble(per_core_outs):
    """[N_CORES][128, P*TILE] -> [B, C, NY, NX] f32."""
    out_full = np.empty((B, C, CELLS_B), np.float32)
    for core in range(N_CORES):
        bb, h = core // 2, core % 2
        o = np.asarray(per_core_outs[core])
        slab = np.concatenate([o[0:64], o[64:128]], axis=1)[:, :HALF]
        out_full[bb, :, h * HALF : (h + 1) * HALF] = slab.astype(np.float32)
    return out_full.reshape(B, C, NY, NX)


def _scatter_fallback(pf, vc):
    """Pure-numpy fallback (exact, last-write-wins) for pathological
    inputs where some 256-cell tile holds more than SLOT pillars."""
    pf = np.asarray(pf, dtype=np.float32)
    vc = np.asarray(vc)
    flat = (
        vc[:, 0].astype(np.int64) * CELLS_B
        + vc[:, 2].astype(np.int64) * NX
        + vc[:, 3].astype(np.int64)
    )
    grid = np.zeros((B * CELLS_B, C), np.float32)
    grid[flat] = pf
    return (
        grid.reshape(B, CELLS_B, C).transpose(0, 2, 1).reshape(B, C, NY, NX)
    )


def _run(pillar_features, voxel_coords, prep=None):
    prep = prep if prep is not None else _host_prep(pillar_features, voxel_coords)
    featAB, offs = prep
    iota = make_iota()

    from concourse.bass_utils import run_bass_kernel_spmd

    nc = _build_bass()
    in_maps = [
        {"featAB": featAB[c], "offs": offs[c], "iota": iota}
        for c in range(N_CORES)
    ]
    res = run_bass_kernel_spmd(nc, in_maps, core_ids=list(range(N_CORES)))
    return _assemble([res.results[c]["out"] for c in range(N_CORES)]), res


def kernel(pillar_features, voxel_coords):
    prep = _host_prep(pillar_features, voxel_coords)
    if prep is None:
        return _scatter_fallback(pillar_features, voxel_coords)
    return _run(pillar_features, voxel_coords, prep=prep)[0]


if __name__ == "__main__":
    # quick numpy-sim self check against last-wins reference
    rng = np.random.default_rng(0)
    n = 100000
    pf = rng.standard_normal((n, 64)).astype(np.float32)
    vc = np.stack(
        [
            rng.integers(0, B, n),
            np.zeros(n, np.int64),
            rng.integers(0, NY, n),
            rng.integers(0, NX, n),
        ],
        axis=1,
    ).astype(np.int64)
    prep = _host_prep(pf, vc)
    assert prep is not None
    featAB, offs = prep
    sim = _assemble([_sim_core(featAB[c], offs[c]) for c in range(N_CORES)])
    ref = _scatter_fallback(pf, vc)
    err = np.abs(sim - ref)
    # fp16 quantization only
    rel = np.linalg.norm((sim - ref).ravel()) / np.linalg.norm(ref.ravel())
    print(f"numpy sim rel err vs f32 last-wins reference: {rel:.3g}")
    assert rel < 1e-3
    print("numpy sim matches (fp16 quantization only)")


# revision 52
# speedup vs baseline: 6.0122x; 6.0122x over previous
"""PointPillarScatter Trainium2 kernel (v2: fp16 + tile pairing).

Strategy: shard by (batch, y-half) -> 8 cores, each producing a
[64, 107136] channel-major slab of the BEV grid, written as fp16 and
upcast to f32 on the host (quantization error ~2.4e-4 L2, far below the
2e-2 gate).

The scatter+transpose+zero-fill is fused into per-pair PE matmuls. Each
core's slab is split into 420 tiles of 256 cells (tile 419 is padding);
tiles t and t+210 are PAIRED so one matmul produces a full 128-partition
output [2 tiles x 64 channels, 256 cells]:

  psum[128, 256] = lhsT[128, 128]^T @ onehot[128, 256]

lhsT is block-diagonal per pair: partitions 0..63 hold tile-A pillar
features in columns 0..63, partitions 64..127 hold tile-B features in
columns 64..127. onehot[k, j] = (cell_offset_k == j) is built by
is_equal against an iota row. PSUM start=True zeros empty cells.

fp16 keeps everything exact: the one-hot is 0/1, each output cell has at
most one contributing pillar (host dedups last-write-wins), offsets
0..255 are exactly representable, so the only error is the single f32 ->
fp16 feature quantization.

Final configuration (CONFIG below):
- x2: each cell offset is stored twice adjacently so the is_equal in0's
  innermost AP dim is a packed [stride 1, size 2] fp16 pair - the shape
  DVE's 2x_1p fast mode needs - while the broadcast sits on a middle AP
  dim and the one-hot stays pair-major (contiguous matmul rhs).
- psum16: matmuls run with is_transpose=True, whose output dtype follows
  lhsT, so PSUM holds fp16; the PSUM->SBUF copies are then 2-byte to
  2-byte (DVE 2x-eligible) and PSUM pipelining is twice as deep.
- feat_mode="diag": the block-diagonal lhsT is expanded host-side and
  DMA'd as one contiguous 128-partition stream per chunk (measured
  faster than compact layouts: DMA cost is per-partition bytes, so two
  64-partition loads cost the same as one 128-partition double-size
  load, and engine-copy assembly just moves the cost elsewhere).
One-hot on DVE, fp16 copies split ACT/DVE by a greedy load balance,
feature loads on the ACT HWDGE ring, output stores batched 8 pairs per
DMA on the SP ring (4KB contiguous per partition).
"""

import numpy as np

B, C, NY, NX = 4, 64, 496, 432
CELLS_B = NY * NX          # 214272 cells per batch
HALF = CELLS_B // 2        # 107136 cells per core slab
N_CORES = 8
TILE = 256                 # cells per tile
P = 210                    # pairs per core; pair q = tiles (q, q+210)
N_T = 2 * P                # 420 tiles (tile 419 is padding)
HALF_PAD = N_T * TILE      # 107520
SLOT = 64                  # pillar slots per tile (max K per 256-cell tile)
GROUP = 8                  # pairs per onehot/stage/output-DMA group
N_GROUPS = (P + GROUP - 1) // GROUP   # 27 (last group has 2 pairs)
CHUNK = 30                 # pairs per feature-DMA chunk (210 = 7*30)
N_CHUNKS = P // CHUNK      # 7
PSUB = 4                   # pairs per PSUM tile ([128, 1024] f32 = 2 banks)


def make_iota(interleave=True):
    """[128, TILE*GROUP] f16.

    interleave=True: column j*GROUP + i holds j — the one-hot is stored
    offset-major / pair-minor so that the is_equal broadcast lands on a
    middle AP dim and every operand's innermost dim is packed stride-1
    fp16, the layout DVE's 2x/4x fast modes need.
    interleave=False: 0..TILE-1 tiled GROUP times (pair-major layout).
    """
    if interleave:
        row = np.repeat(np.arange(TILE, dtype=np.float16), GROUP)
    else:
        row = np.tile(np.arange(TILE, dtype=np.float16), GROUP)
    return np.broadcast_to(row[None, :], (128, TILE * GROUP)).copy()


def _host_prep(pf, vc):
    """Dedup (last-wins), shard, bucket pillars into (core, tile, slot).

    Returns featAB [N_CORES, 128, P*64] f16 (A slots in partitions 0..63
    holding channel columns 0..63 of each pair block... stored compact:
    partition p, pair q, 64 channels), offs [N_CORES, 128, P] f16
    (cell offset within tile, -1 for empty slots), or None if any tile
    has more than SLOT pillars (fallback path).
    """
    pf = np.asarray(pf, dtype=np.float32)
    vc = np.asarray(vc)
    b = vc[:, 0].astype(np.int64)
    y = vc[:, 2].astype(np.int64)
    x = vc[:, 3].astype(np.int64)
    cell = y * NX + x
    key = b * CELLS_B + cell

    # last occurrence of each key wins (matches reference scatter)
    u, idx_rev = np.unique(key[::-1], return_index=True)
    winners = (len(key) - 1) - idx_rev

    wb = u // CELLS_B
    wc = u % CELLS_B
    h = (wc >= HALF).astype(np.int64)
    core = wb * 2 + h
    cl = wc - h * HALF
    t = cl // TILE                     # 0..418
    off = cl % TILE
    half = (t >= P).astype(np.int64)
    q = t - half * P

    # rank within each (core, tile) bucket
    gkey = core * N_T + t
    order = np.argsort(gkey, kind="stable")
    gk_s = gkey[order]
    starts = np.r_[0, np.flatnonzero(np.diff(gk_s)) + 1]
    counts = np.diff(np.r_[starts, len(gk_s)])
    if counts.max() > SLOT:
        return None
    rank = np.arange(len(gk_s)) - np.repeat(starts, counts)

    w_s = winners[order]
    core_s = core[order]
    part_s = half[order] * 64 + rank
    q_s = q[order]
    off_s = off[order]

    feat4 = np.zeros((N_CORES, 128, P, 64), np.float16)
    offs = np.full((N_CORES, 128, P), -1.0, np.float16)
    feat4[core_s, part_s, q_s, :] = pf[w_s].astype(np.float16)
    offs[core_s, part_s, q_s] = off_s.astype(np.float16)
    return feat4.reshape(N_CORES, 128, P * 64), offs


def expand_diag(featAB):
    """[N_CORES, 128, P*64] compact -> [N_CORES, 128, P*128] block-diag."""
    f4 = featAB.reshape(N_CORES, 128, P, 64)
    d = np.zeros((N_CORES, 128, P, 128), np.float16)
    d[:, 0:64, :, 0:64] = f4[:, 0:64]
    d[:, 64:128, :, 64:128] = f4[:, 64:128]
    return d.reshape(N_CORES, 128, P * 128)


def _build_bass(repeat=1, interleave=True, feat_diag=False, probe=None,
                unroll=1, out_split=False, deep_bufs=False, x2=False,
                psum16=False, feat_mode=None):
    # feat_mode: "diag" = block-diag lhsT straight from DRAM (6.9MB);
    # "strided" = compact DRAM, two 64-partition strided DMAs into
    # pre-zeroed buffers; "copy" = compact 128-partition DMA + engine
    # copies build the diagonal blocks; "mm2" = compact DMA, no block
    # diag: two matmuls per pair into partition-offset PSUM halves.
    if feat_mode is None:
        feat_mode = "diag" if feat_diag else "strided"
    feat_diag = feat_mode == "diag"
    import concourse.bacc as bacc
    import concourse.tile as tile
    from concourse import mybir
    from contextlib import ExitStack

    f16 = mybir.dt.float16
    f32 = mybir.dt.float32
    nc = bacc.Bacc("TRN2", target_bir_lowering=False, debug=False)

    fw = 128 if feat_diag else 64
    ow = 2 if x2 else 1
    featAB = nc.dram_tensor("featAB", [128, P * fw], f16, kind="ExternalInput")
    offs = nc.dram_tensor("offs", [128, P * ow], f16, kind="ExternalInput")
    iota = nc.dram_tensor("iota", [128, TILE * GROUP], f16, kind="ExternalInput")
    out = nc.dram_tensor("out", [128, P * TILE], f16, kind="ExternalOutput")

    with tile.TileContext(nc) as tc, ExitStack() as ctx:
        nb = 6 if deep_bufs else 4
        const_p = ctx.enter_context(tc.tile_pool(name="const", bufs=1))
        lhsT_p = ctx.enter_context(tc.tile_pool(name="lhsT", bufs=3))
        oh_p = ctx.enter_context(tc.tile_pool(name="oh", bufs=nb))
        ps_p = ctx.enter_context(
            tc.tile_pool(name="ps", bufs=8 if psum16 else 4, space="PSUM")
        )
        st_p = ctx.enter_context(tc.tile_pool(name="st", bufs=nb))

        iota_t = const_p.tile([128, TILE * GROUP], f16)
        nc.sync.dma_start(out=iota_t[:], in_=iota[:, :])
        if x2:
            off_t = const_p.tile([128, P, 2], f16)
        else:
            off_t = const_p.tile([128, P], f16)
        nc.sync.dma_start(out=off_t[:], in_=offs[:, :])

        # Pre-zero all lhsT chunk buffers once: in-loop DMAs/copies only
        # rewrite the diagonal blocks, so the off-diagonal zeros persist
        # across buffer rotations. (mm2 mode has no diag, needs no zeros.)
        lhsT_w = 64 if feat_mode == "mm2" else 128
        zs = []
        for _ in range(3):
            z = lhsT_p.tile([128, CHUNK, lhsT_w], f16, tag="lhsT")
            if feat_mode != "mm2":
                nc.vector.memset(z[:], 0.0)
            zs.append(z)
        ft_p = ctx.enter_context(tc.tile_pool(name="ft", bufs=3))

        oh_pre = []
        if probe == "no_oh":
            for _ in range(4):
                t = oh_p.tile(
                    [128, TILE, GROUP] if interleave else [128, GROUP * TILE],
                    f16,
                    tag="oh",
                )
                nc.vector.memset(t[:], 0.0)
                oh_pre.append(t)
        st_pre = []
        if probe == "no_copy":
            for _ in range(4):
                t = st_p.tile([128, GROUP * TILE], f16, tag="st")
                nc.vector.memset(t[:], 0.0)
                st_pre.append(t)

        def body():
            chunk_tiles = {}
            state = {"next": 0}
            # greedy engine load balancing (estimated ns per free-element):
            # one-hot runs on DVE (fast-mode eligible); the PSUM->SBUF
            # fp16-converting copies are split between ACT and DVE.
            load = {"dve": 0.0, "act": 0.0}

            def pick(a, b):
                return a if load[a] <= load[b] else b

            def issue_chunks(upto_pair):
                while (
                    state["next"] < N_CHUNKS
                    and state["next"] * CHUNK < upto_pair
                ):
                    c = state["next"]
                    lt = lhsT_p.tile([128, CHUNK, lhsT_w], f16, tag="lhsT")
                    if feat_mode == "diag":
                        lo, hi = c * CHUNK * 128, (c + 1) * CHUNK * 128
                        nc.scalar.dma_start(out=lt[:], in_=featAB[:, lo:hi])
                    elif feat_mode == "strided":
                        lo, hi = c * CHUNK * 64, (c + 1) * CHUNK * 64
                        nc.scalar.dma_start(
                            out=lt[0:64, :, 0:64], in_=featAB[0:64, lo:hi]
                        )
                        nc.scalar.dma_start(
                            out=lt[64:128, :, 64:128], in_=featAB[64:128, lo:hi]
                        )
                    elif feat_mode == "mm2":
                        lo, hi = c * CHUNK * 64, (c + 1) * CHUNK * 64
                        nc.scalar.dma_start(out=lt[:], in_=featAB[:, lo:hi])
                    else:  # "copy": compact 128-partition DMA, then engine
                        # copies place the diagonal blocks
                        ft = ft_p.tile([128, CHUNK, 64], f16, tag="ft")
                        lo, hi = c * CHUNK * 64, (c + 1) * CHUNK * 64
                        nc.scalar.dma_start(out=ft[:], in_=featAB[:, lo:hi])
                        w = pick("act", "dve")
                        load[w] += CHUNK * 64 * (0.83 if w == "act" else 1.04)
                        eng = nc.scalar if w == "act" else nc.vector
                        cp = eng.copy if w == "act" else eng.tensor_copy
                        cp(out=lt[0:64, :, 0:64], in_=ft[0:64, :, :])
                        w = pick("act", "dve")
                        load[w] += CHUNK * 64 * (0.83 if w == "act" else 1.04)
                        eng = nc.scalar if w == "act" else nc.vector
                        cp = eng.copy if w == "act" else eng.tensor_copy
                        cp(out=lt[64:128, :, 64:128], in_=ft[64:128, :, :])
                    chunk_tiles[c] = lt
                    state["next"] += 1

            for g in range(N_GROUPS):
                q0 = g * GROUP
                n = min(GROUP, P - q0)
                if probe != "no_feat":
                    issue_chunks(q0 + n + CHUNK)  # one-chunk lookahead

                # one-hot, offset-major/pair-minor: oh[p, j, i] is 1 iff
                # pillar p of pair q0+i sits at cell offset j. All operands
                # have packed fp16 innermost dims (DVE fast-mode layout).
                if probe == "no_oh":
                    oh = oh_pre[g % 4]
                elif interleave:
                    oh = oh_p.tile([128, TILE, n], f16, tag="oh")
                    load["dve"] += n * TILE * 0.27 + 150
                    nc.vector.tensor_tensor(
                        out=oh[:],
                        in0=off_t[:, q0 : q0 + n]
                        .to_broadcast([128, n, TILE])
                        .transpose([0, 2, 1]),
                        in1=iota_t[:, : TILE * n],
                        op=mybir.AluOpType.is_equal,
                    )
                elif x2:
                    # pair-major one-hot, but in0's innermost AP dim is the
                    # packed duplicated-offset pair [stride 1, size 2] so the
                    # compare qualifies for DVE 2x_1p; the broadcast sits on
                    # a middle dim and the matmul rhs stays contiguous.
                    oh = oh_p.tile([128, n * TILE], f16, tag="oh")
                    load["dve"] += n * TILE * 0.52 + 150
                    nc.vector.tensor_tensor(
                        out=oh[:],
                        in0=off_t[:, q0 : q0 + n, :]
                        .to_broadcast([128, n, 2, TILE // 2])
                        .transpose([0, 1, 3, 2]),
                        in1=iota_t[:, : n * TILE],
                        op=mybir.AluOpType.is_equal,
                    )
                else:
                    oh = oh_p.tile([128, n * TILE], f16, tag="oh")
                    load["dve"] += n * TILE * 1.04 + 150
                    nc.vector.tensor_tensor(
                        out=oh[:],
                        in0=off_t[:, q0 : q0 + n].to_broadcast([128, n, TILE]),
                        in1=iota_t[:, : n * TILE],
                        op=mybir.AluOpType.is_equal,
                    )

                stage = (
                    st_pre[g % 4]
                    if probe == "no_copy"
                    else st_p.tile([128, n * TILE], f16, tag="st")
                )
                for s0 in range(0, n, PSUB):
                    m = min(PSUB, n - s0)
                    psum = ps_p.tile(
                        [128, m * TILE], f16 if psum16 else f32, tag="ps"
                    )
                    if probe == "one_mm":
                        # timing-only probe: one 512-wide matmul per PSUM
                        # bank (wrong data, half the weight loads/insts,
                        # same row count)
                        c, qc = divmod(q0 + s0, CHUNK)
                        for b0 in range(0, m * TILE, 512):
                            w512 = min(512, m * TILE - b0)
                            nc.tensor.matmul(
                                out=psum[:, b0 : b0 + w512],
                                lhsT=chunk_tiles[c][:, qc, :],
                                rhs=oh[:, b0 : b0 + w512],
                                start=True,
                                stop=True,
                                skip_group_check=True,
                            )
                        mm_iter = []
                    else:
                        mm_iter = range(m)
                    for j in mm_iter:
                        qq = q0 + s0 + j
                        c = qq // CHUNK
                        qc = qq % CHUNK
                        src = zs[c % 3] if probe == "no_feat" else chunk_tiles[c]
                        rhs = (
                            oh[:, :, s0 + j]
                            if interleave
                            else oh[:, (s0 + j) * TILE : (s0 + j + 1) * TILE]
                        )
                        if feat_mode == "mm2":
                            for h0 in (0, 64):
                                rhs_h = (
                                    oh[h0 : h0 + 64, :, s0 + j]
                                    if interleave
                                    else oh[
                                        h0 : h0 + 64,
                                        (s0 + j) * TILE : (s0 + j + 1) * TILE,
                                    ]
                                )
                                nc.tensor.matmul(
                                    out=psum[
                                        h0 : h0 + 64,
                                        j * TILE : (j + 1) * TILE,
                                    ],
                                    lhsT=src[h0 : h0 + 64, qc, :],
                                    rhs=rhs_h,
                                    is_transpose=True if psum16 else None,
                                    start=True,
                                    stop=True,
                                )
                        else:
                            nc.tensor.matmul(
                                out=psum[:, j * TILE : (j + 1) * TILE],
                                lhsT=src[:, qc, :],
                                rhs=rhs,
                                is_transpose=True if psum16 else None,
                                start=True,
                                stop=True,
                            )
                    if probe != "no_copy":
                        dve_c = 0.52 if psum16 else 1.04
                        w = pick("act", "dve")
                        load[w] += m * TILE * (0.83 if w == "act" else dve_c) + 300
                        if w == "act":
                            nc.scalar.copy(
                                out=stage[:, s0 * TILE : (s0 + m) * TILE],
                                in_=psum[:],
                            )
                        else:
                            nc.vector.tensor_copy(
                                out=stage[:, s0 * TILE : (s0 + m) * TILE],
                                in_=psum[:],
                            )
                if probe != "no_out":
                    # balance HWDGE rings: scalar carries the feature loads
                    # (~21us), so route every 4th output store there too
                    ring = nc.scalar if out_split and g % 4 == 3 else nc.sync
                    ring.dma_start(
                        out=out[:, q0 * TILE : (q0 + n) * TILE],
                        in_=stage[:, : n * TILE],
                    )

        if repeat == 1:
            for _ in range(unroll):
                body()
        else:
            assert repeat % unroll == 0
            with tc.For_i(0, repeat // unroll, 1):
                for _ in range(unroll):
                    body()

    nc.compile()
    return nc


def _sim_core(featAB_c, offs_c):
    """Numpy simulation of one core's device program (for validation)."""
    fv = featAB_c.reshape(128, P, 64).astype(np.float32)
    out = np.zeros((128, P * TILE), np.float32)
    for q in range(P):
        oh = (
            offs_c[:, q : q + 1].astype(np.float32)
            == np.arange(TILE, dtype=np.float32)[None, :]
        ).astype(np.float32)
        lhsT = np.zeros((128, 128), np.float32)
        lhsT[0:64, 0:64] = fv[0:64, q, :]
        lhsT[64:128, 64:128] = fv[64:128, q, :]
        out[:, q * TILE : (q + 1) * TILE] = lhsT.T @ oh
    return out


def _assemble(per_core_outs):
    """[N_CORES][128, P*TILE] -> [B, C, NY, NX] f32."""
    out_full = np.empty((B, C, CELLS_B), np.float32)
    for core in range(N_CORES):
        bb, h = core // 2, core % 2
        o = np.asarray(per_core_outs[core])
        slab = np.concatenate([o[0:64], o[64:128]], axis=1)[:, :HALF]
        out_full[bb, :, h * HALF : (h + 1) * HALF] = slab.astype(np.float32)
    return out_full.reshape(B, C, NY, NX)


def _scatter_fallback(pf, vc):
    """Pure-numpy fallback (exact, last-write-wins) for pathological
    inputs where some 256-cell tile holds more than SLOT pillars."""
    pf = np.asarray(pf, dtype=np.float32)
    vc = np.asarray(vc)
    flat = (
        vc[:, 0].astype(np.int64) * CELLS_B
        + vc[:, 2].astype(np.int64) * NX
        + vc[:, 3].astype(np.int64)
    )
    grid = np.zeros((B * CELLS_B, C), np.float32)
    grid[flat] = pf
    return (
        grid.reshape(B, CELLS_B, C).transpose(0, 2, 1).reshape(B, C, NY, NX)
    )


CONFIG = {
    "interleave": False,
    "x2": True,
    "psum16": True,
    "feat_mode": "diag",
}


def make_in_maps(featAB, offs, interleave=None, feat_diag=None, x2=None,
                 feat_mode=None):
    if interleave is None:
        interleave = CONFIG.get("interleave", False)
    if x2 is None:
        x2 = CONFIG.get("x2", False)
    if feat_mode is None:
        feat_mode = CONFIG.get("feat_mode")
    if feat_mode is None:
        feat_mode = "diag" if (
            feat_diag if feat_diag is not None
            else CONFIG.get("feat_diag", True)
        ) else "strided"
    feat = expand_diag(featAB) if feat_mode == "diag" else featAB
    iota = make_iota(interleave)
    if x2:
        offs = np.repeat(offs, 2, axis=2).reshape(N_CORES, 128, P * 2)
    return [
        {"featAB": feat[c], "offs": offs[c], "iota": iota}
        for c in range(N_CORES)
    ]


def _run(pillar_features, voxel_coords, prep=None):
    prep = prep if prep is not None else _host_prep(pillar_features, voxel_coords)
    featAB, offs = prep

    from concourse.bass_utils import run_bass_kernel_spmd

    nc = _build_bass(**CONFIG)
    in_maps = make_in_maps(featAB, offs)
    res = run_bass_kernel_spmd(nc, in_maps, core_ids=list(range(N_CORES)))
    return _assemble([res.results[c]["out"] for c in range(N_CORES)]), res


def kernel(pillar_features, voxel_coords):
    prep = _host_prep(pillar_features, voxel_coords)
    if prep is None:
        return _scatter_fallback(pillar_features, voxel_coords)
    return _run(pillar_features, voxel_coords, prep=prep)[0]


if __name__ == "__main__":
    # quick numpy-sim self check against last-wins reference
    rng = np.random.default_rng(0)
    n = 100000
    pf = rng.standard_normal((n, 64)).astype(np.float32)
    vc = np.stack(
        [
            rng.integers(0, B, n),
            np.zeros(n, np.int64),
            rng.integers(0, NY, n),
            rng.integers(0, NX, n),
        ],
        axis=1,
    ).astype(np.int64)
    prep = _host_prep(pf, vc)
    assert prep is not None
    featAB, offs = prep
    sim = _assemble([_sim_core(featAB[c], offs[c]) for c in range(N_CORES)])
    ref = _scatter_fallback(pf, vc)
    err = np.abs(sim - ref)
    # fp16 quantization only
    rel = np.linalg.norm((sim - ref).ravel()) / np.linalg.norm(ref.ravel())
    print(f"numpy sim rel err vs f32 last-wins reference: {rel:.3g}")
    assert rel < 1e-3
    print("numpy sim matches (fp16 quantization only)")
